# revision 1
# baseline (speedup 1.0000x reference)
"""ContraCLM token-level contrastive loss on 8 Trainium2 NeuronCores.

Data-parallel over the batch: core b handles sample b (B=8). Per core,
with S=1536, D=1024, T=0.05:

  f_v = l2norm(h_v) with masked token rows zeroed (mask folded into the
  rsqrt scale);  F = [f1; f2]  (2S x D, bf16, stored transposed as [D, 2S])

  sim = F F^T computed as 24 x 6 grid of [128, 512] PSUM strips (K=1024).
  exp(sim/T) row sums come free from the ScalarE activation free-dim
  accumulator. Diagonal-block strips (self-sim and positive-counterpart
  entries, which land on 128-block diagonals because 2S is a multiple of
  128 and partner offset is S) get the diagonal zeroed via affine_select
  before a DVE row-sum instead.

  Masked columns were zeroed in F, so each masked column contributes
  exp(0)=1 to a row sum: subtract K0 = 2S - 2n afterwards.
  pos_sim is computed exactly in fp32 as a row-wise dot product.
  per_tok = log(Ng + exp(pos_sim/T)) - pos_sim/T; masked mean over 2n
  tokens; AllReduce-mean across the 8 cores.
"""

import sys

for _p in ("/opt/trn_rl_repo", "/opt/pypackages"):
    if _p not in sys.path:
        sys.path.append(_p)

from contextlib import ExitStack

import numpy as np

import bass_rust

import concourse.bass as bass
import concourse.tile as tile
from concourse import mybir
from concourse.bass_utils import run_bass_kernel_spmd
from concourse.masks import make_identity
from concourse.vector_clock import ScopedClock

# The walrus build in this container encodes at most 2 sync waits per
# instruction (bass_rust's inst_waits_full agrees), but Tile's semaphore
# assignment can attach more. Hoist excess waits onto unfusable same-engine
# NoOps immediately before the instruction — the engine executes its queue
# in order, so semantics are preserved.
_MAX_WAITS = 1


def _split_excess_waits(nc, ordered):
    for bb_name, insts in ordered.items():
        out = []
        changed = False
        for inst in insts:
            si = getattr(inst, "sync_info", None)
            waits = list(si.on_wait) if si is not None else []
            if len(waits) > _MAX_WAITS:
                changed = True
                extra, keep = waits[:-_MAX_WAITS], waits[-_MAX_WAITS:]
                for i in range(0, len(extra), _MAX_WAITS):
                    out.append(mybir.InstNoOp(
                        name=nc.get_next_instruction_name(),
                        sync_info=mybir.SyncInfo(
                            on_wait=extra[i:i + _MAX_WAITS], on_update=[]),
                        bass_nofuse=True,
                        engine=inst.engine,
                    ))
                si.on_wait = keep
            out.append(inst)
        if changed:
            insts[:] = out


_orig_lower_ordered_insts = tile.TileContext._lower_ordered_insts


def _patched_lower_ordered_insts(self, ordered):
    _split_excess_waits(self.nc, ordered)
    return _orig_lower_ordered_insts(self, ordered)


tile.TileContext._lower_ordered_insts = _patched_lower_ordered_insts


def _split_waits_drain_and_barrier(self, tick_clock, wait_clock):
    nc = self.nc
    probe = nc.sync.nop(nofuse=True)
    wait_clock.add_sem_waits(
        probe.ins, ScopedClock({None: tick_clock.global_clock}))
    si = probe.ins.sync_info
    waits = list(si.on_wait) if si is not None else []
    if len(waits) > _MAX_WAITS:
        si.on_wait = waits[:_MAX_WAITS]
        for i in range(_MAX_WAITS, len(waits), _MAX_WAITS):
            nxt = nc.sync.nop(nofuse=True)
            nxt.ins.sync_info = bass_rust.SyncInfo(
                on_wait=waits[i:i + _MAX_WAITS], on_update=[])
    nc.sync.drain()
    nc.all_engine_barrier()
    assert self.sems is not None
    popped = nc._tile_sem_poison_stack.pop()
    assert popped is self._sem_poison
    nc.clear_and_free_semaphores(list(self.sems.allocated().values()))
    nc.all_engine_barrier()


tile.TileContext._drain_and_barrier = _split_waits_drain_and_barrier

S, D, NCORES = 1536, 1024, 8
ST = S // 128            # 12 s-tiles per view
NB = 2 * ST              # 24 block rows of F
NCS = 2 * S // 512       # 6 column strips
KT = D // 128            # 8 contraction tiles
TEMP_INV = 20.0          # 1 / 0.05
FP8_SCALE = 8.0          # f entries ~N(0, 1/32); x8 keeps them in e4m3's
                         # normal range (|f|*8 <~ 2, well under 240)
F32 = mybir.dt.float32
BF16 = mybir.dt.bfloat16
FP8 = mybir.dt.float8e4
AF = mybir.ActivationFunctionType
ALU = mybir.AluOpType


def _build(num_devices: int = NCORES, debug_dump: bool = False) -> bass.Bass:
    nc = bass.Bass(num_devices=num_devices)
    h1 = nc.dram_tensor("h1", [S, D], F32, kind="ExternalInput")
    h2 = nc.dram_tensor("h2", [S, D], F32, kind="ExternalInput")
    # mask, pre-laid-out host-side as [128, ST] so token t = 128*col + row
    maskT = nc.dram_tensor("maskT", [128, ST], F32, kind="ExternalInput")
    out = nc.dram_tensor("loss", [1, 1], F32, kind="ExternalOutput")
    if debug_dump:
        ng_dump = nc.dram_tensor("ng_dump", [128, NB], F32,
                                 kind="ExternalOutput")
        cacc_dump = nc.dram_tensor("cacc_dump", [128, ST], F32,
                                   kind="ExternalOutput")

    exp_scale = TEMP_INV / (FP8_SCALE * FP8_SCALE)

    with tile.TileContext(nc) as tc, ExitStack() as ctx:
        const_pool = ctx.enter_context(tc.tile_pool(name="const", bufs=1))
        big = ctx.enter_context(tc.tile_pool(name="big", bufs=1))
        stat = ctx.enter_context(tc.tile_pool(name="stat", bufs=1))

        ident = const_pool.tile([128, 128], BF16)
        make_identity(nc, ident[:])
        ones_col = const_pool.tile([128, 1], F32)
        nc.gpsimd.memset(ones_col[:], 1.0)
        ones_sq = const_pool.tile([128, 128], F32)
        nc.gpsimd.memset(ones_sq[:], 1.0)
        ones_bf = const_pool.tile([128, 1], BF16)
        nc.gpsimd.memset(ones_bf[:], 1.0)
        msk = const_pool.tile([128, ST], F32)
        nc.sync.dma_start(msk[:], maskT[:])

        fT1 = big.tile([128, KT, S], FP8)        # f1^T * 8, fp8e4
        fT2 = big.tile([128, KT, S], FP8)        # f2^T * 8
        h1keep = big.tile([128, ST, D], F32)     # raw h1, kept for pos dot
        s12 = stat.tile([128, ST], F32)          # raw <h1_i, h2_i>
        poss = stat.tile([128, ST], F32)         # pos_sim (masked rows -> 0)
        sc1buf = stat.tile([128, ST], F32)       # masked rsqrt scales view1
        acc = stat.tile([128, NB, NCS], F32)     # per-strip row sums
        cacc = stat.tile([128, ST], F32)         # B column sums (view-2 Ng)
        msk24 = stat.tile([128, NB], F32)
        pose24 = stat.tile([128, NB], F32)       # exp(pos_sim/T), doubled
        poss20m = stat.tile([128, NB], F32)      # mask * pos_sim/T, doubled
        negK0 = stat.tile([128, 1], F32)
        recn = stat.tile([1, 1], F32)

        # acc[view2 rows, A-col-strips] is never written; zero everything
        nc.gpsimd.memset(acc[:], 0.0)

        # ---- phase 0: mask-only precomputes ----
        with tc.tile_pool(name="ep0", bufs=1) as ep0, \
             tc.tile_pool(name="ep0_ps", bufs=1, space="PSUM") as ep0p:
            msum = ep0.tile([128, 1], F32)
            nc.vector.tensor_reduce(msum[:], msk[:],
                                    axis=mybir.AxisListType.X, op=ALU.add)
            nps = ep0p.tile([128, 1], F32)
            nc.tensor.matmul(nps[:], ones_sq[:], msum[:], start=True,
                             stop=True)
            # -K0 = 2n - 2S
            nc.scalar.activation(negK0[:], nps[:], AF.Copy, scale=2.0,
                                 bias=float(-2 * S))
            n2c = ep0.tile([1, 1], F32)
            nc.scalar.activation(n2c[:], nps[0:1, :], AF.Copy,
                                 scale=float(2 * num_devices))
            nc.vector.reciprocal(recn[:], n2c[:])   # 1/(2 n ncores)
            nc.vector.tensor_copy(msk24[:, 0:ST], msk[:])
            nc.vector.tensor_copy(msk24[:, ST:NB], msk[:])

        def load_view(t, dst, src_dram, keep):
            if keep is not None:
                ht = keep
                nc.sync.dma_start(ht[:], src_dram[t * 128:(t + 1) * 128, :])
            else:
                nc.sync.dma_start(dst[:], src_dram[t * 128:(t + 1) * 128, :])
                ht = dst
            return ht

        def norms_of(scp, scr, ht, t, tag):
            sq = scr.tile([128, D], BF16, tag="sq", name=f"sq_{tag}_{t}")
            ss = scp.tile([128, 1], F32, tag="ss", name=f"ss_{tag}_{t}")
            nc.scalar.activation(sq[:], ht[:], AF.Square, accum_out=ss[:])
            nrm = scp.tile([128, 1], F32, tag="nrm", name=f"nrm_{tag}_{t}")
            nc.scalar.sqrt(nrm[:], ss[:])
            ri = scp.tile([128, 1], F32, tag="ri", name=f"ri_{tag}_{t}")
            nc.vector.reciprocal(ri[:], nrm[:])
            sc = scp.tile([128, 1], F32, tag="msc", name=f"sc_{tag}_{t}")
            nc.vector.tensor_mul(sc[:], ri[:], msk[:, t:t + 1])
            return sc

        def normalize_transpose(scr, tps, ht, sc, fT, t, tag):
            fn = scr.tile([128, D], BF16, tag="fn", name=f"fn_{tag}_{t}")
            nc.vector.tensor_scalar_mul(fn[:], ht[:], sc[:])
            c0 = t * 128
            for kg in range(2):
                pt = tps.tile([128, 512], BF16, tag="pt", name=f"pt_{tag}_{t}_{kg}")
                for j in range(4):
                    k = kg * 4 + j
                    nc.tensor.transpose(pt[:, j * 128:(j + 1) * 128],
                                        fn[:, k * 128:(k + 1) * 128],
                                        ident[:])
                nc.vector.tensor_scalar_mul(
                    fT[:, kg * 4:(kg + 1) * 4, c0:c0 + 128],
                    pt[:].rearrange("p (j c) -> p j c", j=4),
                    FP8_SCALE)

        def strip(mmp, esp, cs, r):
            """One [128,512] sim strip: matmuls, exp, row-sum into acc."""
            lhsT = fT1 if r < ST else fT2
            rT = r % ST
            rhsT = fT1 if cs < NCS // 2 else fT2
            csT = cs % (NCS // 2)
            ps = mmp.tile([128, 512], F32, tag="ps", name=f"ps_{cs}_{r}")
            for g in range(KT // 2):
                nc.tensor.matmul(
                    ps[:],
                    lhsT[:, 2 * g:2 * g + 2, rT * 128:(rT + 1) * 128],
                    rhsT[:, 2 * g:2 * g + 2, csT * 512:(csT + 1) * 512],
                    perf_mode=mybir.MatmulPerfMode.DoubleRow,
                    start=(g == 0), stop=(g == KT // 2 - 1))
            es = esp.tile([128, 512], BF16, tag="es", name=f"es_{cs}_{r}")
            bad = [bc for bc in (r % ST, r % ST + ST)
                   if cs * 4 <= bc < cs * 4 + 4]
            if bad:
                jb = bad[0] - cs * 4
                nc.scalar.activation(es[:], ps[:], AF.Exp, scale=exp_scale)
                blk = es[:, jb * 128:(jb + 1) * 128]
                nc.gpsimd.affine_select(
                    out=blk, in_=blk, compare_op=ALU.not_equal,
                    fill=0.0, base=0, pattern=[[-1, 128]],
                    channel_multiplier=1)
                nc.vector.tensor_reduce(acc[:, r, cs:cs + 1], es[:],
                                        axis=mybir.AxisListType.X,
                                        op=ALU.add)
            else:
                nc.scalar.activation(es[:], ps[:], AF.Exp, scale=exp_scale,
                                     accum_out=acc[:, r, cs:cs + 1])
            return es

        with tc.tile_pool(name="mm_ps", bufs=3, space="PSUM") as mmp, \
             tc.tile_pool(name="es", bufs=3) as esp, \
             tc.tile_pool(name="scr", bufs=2) as scr, \
             tc.tile_pool(name="sc", bufs=4) as scp:

            # ---- phase A: view-1 load/normalize/transpose ----
            with tc.tile_pool(name="tpA_ps", bufs=3, space="PSUM") as tps:
                for t in range(ST):
                    ht = load_view(t, None, h1, h1keep[:, t, :])
                    sc1 = norms_of(scp, scr, ht, t, "a")
                    nc.vector.tensor_copy(sc1buf[:, t:t + 1], sc1[:])
                    normalize_transpose(scr, tps, ht, sc1, fT1, t, "a")

            # ---- phase A': A-quadrant strips (only need view 1) ----
            for cs in range(NCS // 2):
                for r in range(ST):
                    strip(mmp, esp, cs, r)

            # ---- phase B: view-2 load/normalize/transpose + pos dot ----
            with tc.tile_pool(name="tpB_ps", bufs=3, space="PSUM") as tps, \
                 tc.tile_pool(name="ldB", bufs=3) as ldB:
                for t in range(ST):
                    tb = ldB.tile([128, D], F32, tag="h2", name=f"h2_{t}")
                    load_view(t, tb, h2, None)
                    sc2 = norms_of(scp, scr, tb, t, "b")
                    prod = scr.tile([128, D], F32, tag="prod",
                                    name=f"prod_{t}")
                    nc.vector.tensor_mul(prod[:], h1keep[:, t, :], tb[:])
                    nc.vector.tensor_reduce(s12[:, t:t + 1], prod[:],
                                            axis=mybir.AxisListType.X,
                                            op=ALU.add)
                    ptmp = scp.tile([128, 1], F32, tag="ptmp",
                                    name=f"ptmp_{t}")
                    nc.vector.tensor_mul(ptmp[:], s12[:, t:t + 1],
                                         sc1buf[:, t:t + 1])
                    nc.vector.tensor_mul(poss[:, t:t + 1], ptmp[:], sc2[:])
                    normalize_transpose(scr, tps, tb, sc2, fT2, t, "b")

            # pos-dependent epilogue precomputes (overlap with B/C strips)
            nc.scalar.activation(pose24[:, 0:ST], poss[:], AF.Exp,
                                 scale=TEMP_INV)
            nc.scalar.activation(pose24[:, ST:NB], poss[:], AF.Exp,
                                 scale=TEMP_INV)
            p20 = stat.tile([128, ST], F32)
            nc.scalar.mul(p20[:], poss[:], TEMP_INV)
            nc.vector.tensor_mul(poss20m[:, 0:ST], p20[:], msk[:])
            nc.vector.tensor_copy(poss20m[:, ST:NB], poss20m[:, 0:ST])

            # ---- phase B': B and C strips + B column sums ----
            with tc.tile_pool(name="cb_ps", bufs=1, space="PSUM") as cbp:
                for cs in range(NCS // 2, NCS):
                    pcb = []
                    for jb in range(4):
                        pcb_jb = cbp.tile([128, 1], F32, tag=f"cb{jb}",
                                          name=f"pcb_{cs}_{jb}")
                        pcb.append(pcb_jb)
                    for r in range(NB):
                        es = strip(mmp, esp, cs, r)
                        if r < ST:
                            for jb in range(4):
                                nc.tensor.matmul(
                                    pcb[jb][:],
                                    es[:, jb * 128:(jb + 1) * 128],
                                    ones_bf[:],
                                    start=(r == 0), stop=(r == ST - 1),
                                    skip_group_check=True)
                    c0 = (cs - NCS // 2) * 4
                    for jb in range(4):
                        nc.vector.tensor_copy(cacc[:, c0 + jb:c0 + jb + 1],
                                              pcb[jb][:])

        # ---- phase C: final reduction chain ----
        with tc.tile_pool(name="ep", bufs=1) as ep, \
             tc.tile_pool(name="ep_ps", bufs=1, space="PSUM") as epp:
            ng = ep.tile([128, NB], F32)
            nc.vector.tensor_reduce(ng[:], acc[:], axis=mybir.AxisListType.X,
                                    op=ALU.add)
            nc.vector.tensor_add(ng[:, ST:NB], ng[:, ST:NB], cacc[:])
            if debug_dump:
                nc.sync.dma_start(ng_dump[:], ng[:])
                nc.sync.dma_start(cacc_dump[:], cacc[:])
            denom = ep.tile([128, NB], F32)
            nc.vector.tensor_scalar_add(denom[:], ng[:], negK0[:])
            nc.vector.tensor_add(denom[:], denom[:], pose24[:])
            lg = ep.tile([128, NB], F32)
            nc.scalar.activation(lg[:], denom[:], AF.Ln)
            ptok = ep.tile([128, NB], F32)
            nc.vector.tensor_mul(ptok[:], lg[:], msk24[:])
            nc.vector.tensor_sub(ptok[:], ptok[:], poss20m[:])
            tsum = ep.tile([128, 1], F32)
            nc.vector.tensor_reduce(tsum[:], ptok[:],
                                    axis=mybir.AxisListType.X, op=ALU.add)
            lps = epp.tile([1, 1], F32)
            nc.tensor.matmul(lps[:], ones_col[:], tsum[:], start=True,
                             stop=True)
            lsb = ep.tile([1, 1], F32)
            nc.vector.tensor_mul(lsb[:], lps[:], recn[:])

            with tc.tile_pool(name="dram", bufs=1, space="DRAM") as dram:
                if num_devices > 1:
                    lin = dram.tile([1, 1], F32)
                    lout = dram.tile([1, 1], F32)
                    nc.sync.dma_start(lin[:], lsb[:])
                    nc.gpsimd.collective_compute(
                        "AllReduce", ALU.add,
                        replica_groups=[list(range(num_devices))],
                        ins=[lin.opt()], outs=[lout.opt()])
                    nc.sync.dma_start(out[:], lout[:])
                else:
                    nc.sync.dma_start(out[:], lsb[:])

    return nc


_NC = None


def _mask_layout(mask_row: np.ndarray) -> np.ndarray:
    # token t = 128 * col + row  ->  [128, ST]
    return np.ascontiguousarray(
        mask_row.astype(np.float32).reshape(ST, 128).T)


def kernel(last_hidden_states_1, last_hidden_states_2, token_mask_batch):
    global _NC
    h1 = np.ascontiguousarray(np.asarray(last_hidden_states_1,
                                         dtype=np.float32))
    h2 = np.ascontiguousarray(np.asarray(last_hidden_states_2,
                                         dtype=np.float32))
    mask = np.asarray(token_mask_batch)
    assert h1.shape == (NCORES, S, D), h1.shape

    if _NC is None:
        _NC = _build(NCORES)

    in_maps = [
        {"h1": h1[b], "h2": h2[b], "maskT": _mask_layout(mask[b])}
        for b in range(NCORES)
    ]
    res = run_bass_kernel_spmd(_NC, in_maps, list(range(NCORES)))
    loss = np.asarray(res.results[0]["loss"], dtype=np.float32).reshape(())
    return loss



# revision 10
# speedup vs baseline: 1.1320x; 1.1320x over previous
"""ContraCLM token-level contrastive loss on 8 Trainium2 NeuronCores.

Data-parallel over the batch: core b handles sample b (B=8). Per core,
with S=1536, D=1024, T=0.05:

  sum-of-squares per token via DVE tensor_tensor_reduce (h*h, add);
  one Sqrt per view + DVE reciprocal gives the masked rsqrt scale.
  The scale (x8 for fp8 range) is folded into the PE transpose by using
  diag(scale) as the transpose moving operand, so the raw f32 tiles are
  transposed+normalized in one pass: fT = [D, 2S] fp8e4 of 8*f.

  sim strips F F^T via fp8 DoubleRow matmuls (K=1024 in 4 groups).
  exp(sim/T) row sums come from the ScalarE activation free-dim
  accumulator.  Self-similarity diagonals get -1e9 added in PSUM
  (DVE + negI constant) BEFORE exp, so exp gives exactly 0.

  Positive-counterpart diagonals stay IN the row sum: denom = Ng + pos
  is just the full row sum (self excluded).  pos_sim itself is read off
  the B-quadrant diagonal via one tensor_tensor_reduce against identity.

  Masked columns contribute exp(0)=1 each: subtract K0 = 2S - 2n.
  per_tok = log(denom + K0') - pos_sim/T; masked mean over 2n tokens.
  Each core returns its per-sample mean; the host averages the 8
  scalars (no device collective).

  View-2 rows' cross-quadrant sums (C = B^T) come from column sums of
  the B strips: es strips are accumulated into a bf16 column
  accumulator on DVE, then 12 tiny per-block matmuls against ones fold
  the partition dimension at the end.
"""

import sys

for _p in ("/opt/trn_rl_repo", "/opt/pypackages"):
    if _p not in sys.path:
        sys.path.append(_p)

from contextlib import ExitStack

import numpy as np

import bass_rust

import concourse.bass as bass
import concourse.tile as tile
from concourse import mybir
from concourse.bass_utils import run_bass_kernel_spmd
from concourse.masks import make_identity
from concourse.vector_clock import ScopedClock

# The walrus build in this container encodes at most 2 sync waits per
# instruction (bass_rust's inst_waits_full agrees), but Tile's semaphore
# assignment can attach more. Hoist excess waits onto unfusable same-engine
# NoOps immediately before the instruction — the engine executes its queue
# in order, so semantics are preserved.
_MAX_WAITS = 1


def _split_excess_waits(nc, ordered):
    for bb_name, insts in ordered.items():
        out = []
        changed = False
        for inst in insts:
            si = getattr(inst, "sync_info", None)
            waits = list(si.on_wait) if si is not None else []
            if len(waits) > _MAX_WAITS:
                changed = True
                extra, keep = waits[:-_MAX_WAITS], waits[-_MAX_WAITS:]
                for i in range(0, len(extra), _MAX_WAITS):
                    out.append(mybir.InstNoOp(
                        name=nc.get_next_instruction_name(),
                        sync_info=mybir.SyncInfo(
                            on_wait=extra[i:i + _MAX_WAITS], on_update=[]),
                        bass_nofuse=True,
                        engine=inst.engine,
                    ))
                si.on_wait = keep
            out.append(inst)
        if changed:
            insts[:] = out


_orig_lower_ordered_insts = tile.TileContext._lower_ordered_insts


def _patched_lower_ordered_insts(self, ordered):
    _split_excess_waits(self.nc, ordered)
    return _orig_lower_ordered_insts(self, ordered)


tile.TileContext._lower_ordered_insts = _patched_lower_ordered_insts


def _split_waits_drain_and_barrier(self, tick_clock, wait_clock):
    nc = self.nc
    probe = nc.sync.nop(nofuse=True)
    wait_clock.add_sem_waits(
        probe.ins, ScopedClock({None: tick_clock.global_clock}))
    si = probe.ins.sync_info
    waits = list(si.on_wait) if si is not None else []
    if len(waits) > _MAX_WAITS:
        si.on_wait = waits[:_MAX_WAITS]
        for i in range(_MAX_WAITS, len(waits), _MAX_WAITS):
            nxt = nc.sync.nop(nofuse=True)
            nxt.ins.sync_info = bass_rust.SyncInfo(
                on_wait=waits[i:i + _MAX_WAITS], on_update=[])
    nc.sync.drain()
    nc.all_engine_barrier()
    assert self.sems is not None
    popped = nc._tile_sem_poison_stack.pop()
    assert popped is self._sem_poison
    nc.clear_and_free_semaphores(list(self.sems.allocated().values()))
    nc.all_engine_barrier()


tile.TileContext._drain_and_barrier = _split_waits_drain_and_barrier

S, D, NCORES = 1536, 1024, 8
ST = S // 128            # 12 s-tiles per view
NB = 2 * ST              # 24 block rows of F
KT = D // 128            # 8 contraction tiles
TEMP_INV = 20.0          # 1 / 0.05
FP8_SCALE = 8.0          # f entries ~N(0, 1/32); x8 keeps them in e4m3's
                         # normal range (|f|*8 <~ 2, well under 240)
EXP_SCALE = TEMP_INV / (FP8_SCALE * FP8_SCALE)
F32 = mybir.dt.float32
BF16 = mybir.dt.bfloat16
FP8 = mybir.dt.float8e4
AF = mybir.ActivationFunctionType
ALU = mybir.AluOpType
DR = mybir.MatmulPerfMode.DoubleRow


def _build(num_devices: int = NCORES, debug_dump: bool = False) -> bass.Bass:
    nc = bass.Bass(num_devices=num_devices)
    h1 = nc.dram_tensor("h1", [S, D], F32, kind="ExternalInput")
    h2 = nc.dram_tensor("h2", [S, D], F32, kind="ExternalInput")
    # mask, pre-laid-out host-side as [128, ST] so token t = 128*col + row
    maskT = nc.dram_tensor("maskT", [128, ST], F32, kind="ExternalInput")
    out = nc.dram_tensor("loss", [1, 1], F32, kind="ExternalOutput")
    if debug_dump:
        ng_dump = nc.dram_tensor("ng_dump", [128, NB], F32,
                                 kind="ExternalOutput")
        poss_dump = nc.dram_tensor("poss_dump", [128, ST], F32,
                                   kind="ExternalOutput")
        sc8_dump = nc.dram_tensor("sc8_dump", [128, NB], F32,
                                  kind="ExternalOutput")
        acc_dump = nc.dram_tensor("acc_dump", [128, NB * 3], F32,
                                  kind="ExternalOutput")
        cac_dump = nc.dram_tensor("cac_dump", [128, S], F32,
                                  kind="ExternalOutput")
        fT_dump = nc.dram_tensor("fT_dump", [128, KT * 128], F32,
                                 kind="ExternalOutput")

    with tile.TileContext(nc) as tc, ExitStack() as ctx:
        const_pool = ctx.enter_context(tc.tile_pool(name="const", bufs=1))
        big = ctx.enter_context(tc.tile_pool(name="big", bufs=1))
        stat = ctx.enter_context(tc.tile_pool(name="stat", bufs=1))

        h1k = big.tile([128, ST, D], F32)
        h2k = big.tile([128, ST, D], F32)
        fT1 = big.tile([128, KT, S], FP8)        # f1^T * 8, fp8e4
        fT2 = big.tile([128, KT, S], FP8)        # f2^T * 8

        msk = const_pool.tile([128, ST], F32)
        # input DMAs first: they are the long pole at startup
        nc.sync.dma_start(msk[:], maskT[:])
        for t in range(ST):
            nc.sync.dma_start(h1k[:, t, :], h1[t * 128:(t + 1) * 128, :])
        for t in range(ST):
            nc.sync.dma_start(h2k[:, t, :], h2[t * 128:(t + 1) * 128, :])

        identF = const_pool.tile([128, 128], F32)
        make_identity(nc, identF[:])
        identB = const_pool.tile([128, 128], BF16)
        make_identity(nc, identB[:])
        negI = const_pool.tile([128, 128], F32)
        nc.gpsimd.memset(negI[:], 0.0)
        nc.gpsimd.affine_select(
            out=negI[:], in_=negI[:], compare_op=ALU.not_equal,
            fill=-1e9, base=0, pattern=[[-1, 128]], channel_multiplier=1)
        ones_col = const_pool.tile([128, 1], F32)
        nc.gpsimd.memset(ones_col[:], 1.0)
        ones_sq = const_pool.tile([128, 128], F32)
        nc.gpsimd.memset(ones_sq[:], 1.0)
        ones_bf = const_pool.tile([128, 1], BF16)
        nc.gpsimd.memset(ones_bf[:], 1.0)

        ss = stat.tile([128, NB], F32)           # per-token sum of squares
        sc8 = stat.tile([128, NB], F32)          # 8 * mask * rsqrt(ss)
        nrm = stat.tile([128, NB], F32)
        acc = stat.tile([128, NB, 3], F32)       # per-strip row sums
        cac = stat.tile([128, S], BF16)          # B col accumulator (bf16)
        poss20 = stat.tile([128, ST], F32)       # pos_sim / T
        msk24 = stat.tile([128, NB], F32)
        negK0 = stat.tile([128, 1], F32)
        recn = stat.tile([1, 1], F32)

        nc.gpsimd.memset(acc[:], 0.0)
        nc.gpsimd.memset(cac[:], 0.0)

        # ---- mask-only precomputes ----
        with tc.tile_pool(name="ep0", bufs=1) as ep0, \
             tc.tile_pool(name="ep0_ps", bufs=1, space="PSUM") as ep0p:
            msum = ep0.tile([128, 1], F32)
            nc.vector.tensor_reduce(msum[:], msk[:],
                                    axis=mybir.AxisListType.X, op=ALU.add)
            nps = ep0p.tile([128, 1], F32)
            nc.tensor.matmul(nps[:], ones_sq[:], msum[:], start=True,
                             stop=True)
            # -K0 = 2n - 2S
            nc.scalar.activation(negK0[:], nps[:], AF.Copy, scale=2.0,
                                 bias=float(-2 * S))
            n2c = ep0.tile([1, 1], F32)
            nc.scalar.activation(n2c[:], nps[0:1, :], AF.Copy, scale=2.0)
            nc.vector.reciprocal(recn[:], n2c[:])   # 1 / (2n)
            nc.vector.tensor_copy(msk24[:, 0:ST], msk[:])
            nc.vector.tensor_copy(msk24[:, ST:NB], msk[:])

        def norms(hk, half, sqp):
            """ss[:, half*ST + t] = sum_d hk[:,t,d]^2, then scale."""
            o = half * ST
            for t in range(ST):
                sq = sqp.tile([128, D], BF16, tag="sq", name=f"sq_{half}_{t}")
                nc.scalar.activation(sq[:], hk[:, t, :], AF.Square,
                                     accum_out=ss[:, o + t:o + t + 1])
            nc.scalar.activation(nrm[:, o:o + ST], ss[:, o:o + ST], AF.Sqrt)
            ri = sqp.tile([128, ST], F32, tag="ri", name=f"ri_{half}")
            nc.vector.reciprocal(ri[:], nrm[:, o:o + ST])
            rm = sqp.tile([128, ST], F32, tag="rm", name=f"rm_{half}")
            nc.vector.tensor_mul(rm[:], ri[:], msk[:])
            nc.vector.tensor_scalar_mul(sc8[:, o:o + ST], rm[:], FP8_SCALE)

        def transpose_view(hk, fT, half, tps, scr):
            """fT[:, k, t*128+c] = hk[c, t, k*128+p] * sc8 via bf16
            normalize (DVE) + PE transpose + plain psum->fp8 copies."""
            o = half * ST
            for t in range(ST):
                fn = scr.tile([128, D], BF16, tag="fn",
                              name=f"fn_{half}_{t}")
                nc.vector.tensor_scalar_mul(fn[:], hk[:, t, :],
                                            sc8[:, o + t:o + t + 1])
                c0 = t * 128
                for kg in range(2):
                    pt = tps.tile([128, 512], BF16, tag="pt",
                                  name=f"pt_{half}_{t}_{kg}")
                    for j in range(4):
                        k = kg * 4 + j
                        nc.tensor.transpose(pt[:, j * 128:(j + 1) * 128],
                                            fn[:, k * 128:(k + 1) * 128],
                                            identB[:])
                    dst = fT[:, kg * 4:(kg + 1) * 4, c0:c0 + 128]
                    src = pt[:].rearrange("p (j c) -> p j c", j=4)
                    if kg == 0:
                        nc.vector.tensor_copy(dst, src)
                    else:
                        nc.scalar.copy(dst, src)

        def mm_strip(ps, ncols, lhsT, rT, rhsT, col0):
            """[128, ncols] sim strip into psum ps (DoubleRow, K=1024)."""
            nsub = ncols // 512
            for g in range(KT // 2):
                for u in range(nsub):
                    nc.tensor.matmul(
                        ps[:, u * 512:(u + 1) * 512],
                        lhsT[:, 2 * g:2 * g + 2, rT * 128:(rT + 1) * 128],
                        rhsT[:, 2 * g:2 * g + 2,
                             col0 + u * 512:col0 + (u + 1) * 512],
                        perf_mode=DR,
                        start=(g == 0), stop=(g == KT // 2 - 1))

        # ---- phase A: view-1 norms + normalized transpose ----
        with tc.tile_pool(name="sqpA", bufs=2) as sqp, \
             tc.tile_pool(name="scrA", bufs=2) as scr, \
             tc.tile_pool(name="tpA_ps", bufs=2, space="PSUM") as tps:
            norms(h1k, 0, sqp)
            transpose_view(h1k, fT1, 0, tps, scr)

        # ---- phase A': A-quadrant strips (view1 x view1) ----
        # pair strip [128,1024] (cols 0..1023) + single [128,512]
        with tc.tile_pool(name="psA", bufs=2, space="PSUM") as psA, \
             tc.tile_pool(name="esA", bufs=2) as esA, \
             tc.tile_pool(name="dvA", bufs=2) as dvA:
            for r in range(ST):
                pair = psA.tile([128, 1024], F32, tag="pp", name=f"pp_{r}")
                mm_strip(pair, 1024, fT1, r, fT1, 0)
                sing = psA.tile([128, 512], F32, tag="psg", name=f"ps_{r}")
                mm_strip(sing, 512, fT1, r, fT1, 1024)
                # self diagonal -> -1e9 before exp
                if r < 8:
                    blk = pair[:, r * 128:(r + 1) * 128]
                else:
                    blk = sing[:, (r - 8) * 128:(r - 7) * 128]
                nc.vector.tensor_add(blk, blk, negI[:])
                ep = esA.tile([128, 1024], BF16, tag="ep", name=f"ep_{r}")
                nc.scalar.activation(ep[:], pair[:], AF.Exp, scale=EXP_SCALE,
                                     accum_out=acc[:, r, 0:1])
                es = esA.tile([128, 512], BF16, tag="es", name=f"es_{r}")
                nc.scalar.activation(es[:], sing[:], AF.Exp, scale=EXP_SCALE,
                                     accum_out=acc[:, r, 1:2])

        # ---- phase B: view-2 norms + normalized transpose ----
        with tc.tile_pool(name="sqpB", bufs=2) as sqp, \
             tc.tile_pool(name="scrB", bufs=2) as scr, \
             tc.tile_pool(name="tpB_ps", bufs=2, space="PSUM") as tps:
            norms(h2k, 1, sqp)
            transpose_view(h2k, fT2, 1, tps, scr)

        # ---- phase B': B and D quadrant strips (cols = view2) ----
        with tc.tile_pool(name="psB", bufs=2, space="PSUM") as psB, \
             tc.tile_pool(name="cb_ps", bufs=1, space="PSUM") as cbp, \
             tc.tile_pool(name="esB", bufs=2) as esB, \
             tc.tile_pool(name="dvB", bufs=2) as dvB:
            pcb = cbp.tile([128, ST], F32)
            for r in range(NB):
                rT = r % ST
                lhsT = fT1 if r < ST else fT2
                trip = psB.tile([128, 1536], F32, tag="tp", name=f"tp_{r}")
                mm_strip(trip, 1536, lhsT, rT, fT2, 0)
                if r < ST:
                    # positive-counterpart diagonal: extract 64*pos_sim,
                    # keep it inside the row sum
                    dscr = dvB.tile([128, 128], F32, tag="dg",
                                    name=f"dg_{r}")
                    nc.vector.tensor_mul(
                        dscr[:], trip[:, rT * 128:(rT + 1) * 128],
                        identF[:])
                    nc.vector.tensor_reduce(
                        poss20[:, rT:rT + 1], dscr[:],
                        axis=mybir.AxisListType.X, op=ALU.add)
                else:
                    blk = trip[:, rT * 128:(rT + 1) * 128]
                    nc.vector.tensor_add(blk, blk, negI[:])
                es = esB.tile([128, 1536], BF16, tag="es", name=f"esb_{r}")
                nc.scalar.activation(es[:], trip[:], AF.Exp, scale=EXP_SCALE,
                                     accum_out=acc[:, r, 2:3])
                if r < ST:
                    nc.vector.tensor_add(cac[:], cac[:], es[:])
                if r == ST - 1:
                    # fold partition dim of cac: col sums for view-2 rows
                    for jb in range(ST):
                        nc.tensor.matmul(
                            pcb[:, jb:jb + 1],
                            cac[:, jb * 128:(jb + 1) * 128],
                            ones_bf[:], start=True, stop=True,
                            skip_group_check=True)

            # ---- epilogue ----
            with tc.tile_pool(name="ep", bufs=1) as ep, \
                 tc.tile_pool(name="ep_ps", bufs=1, space="PSUM") as epp:
                ng = ep.tile([128, NB], F32)
                nc.vector.tensor_reduce(ng[:], acc[:],
                                        axis=mybir.AxisListType.X,
                                        op=ALU.add)
                nc.vector.tensor_add(ng[:, ST:NB], ng[:, ST:NB], pcb[:])
                denom = ep.tile([128, NB], F32)
                nc.vector.tensor_scalar_add(denom[:], ng[:], negK0[:])
                lg = ep.tile([128, NB], F32)
                nc.scalar.activation(lg[:], denom[:], AF.Ln)
                if debug_dump:
                    nc.sync.dma_start(ng_dump[:], ng[:])
                    nc.sync.dma_start(poss_dump[:], poss20[:])
                    nc.sync.dma_start(sc8_dump[:], sc8[:])
                    nc.sync.dma_start(
                        acc_dump[:],
                        acc[:].rearrange("p a b -> p (a b)"))
                    cacf = ep.tile([128, S], F32)
                    nc.vector.tensor_copy(cacf[:], cac[:])
                    nc.sync.dma_start(cac_dump[:], cacf[:])
                    fTf = ep.tile([128, KT, 128], F32)
                    nc.vector.tensor_copy(fTf[:], fT1[:, :, 0:128])
                    nc.sync.dma_start(
                        fT_dump[:],
                        fTf[:].rearrange("p a b -> p (a b)"))
                ptok = ep.tile([128, NB], F32)
                nc.vector.tensor_mul(ptok[:], lg[:], msk24[:])
                p20m = ep.tile([128, ST], F32)
                nc.vector.tensor_mul(p20m[:], poss20[:], msk[:])
                # poss20 held 64*pos_sim (raw psum); scale to pos_sim/T
                nc.vector.tensor_scalar_mul(p20m[:], p20m[:], EXP_SCALE)
                nc.vector.tensor_sub(ptok[:, 0:ST], ptok[:, 0:ST], p20m[:])
                nc.vector.tensor_sub(ptok[:, ST:NB], ptok[:, ST:NB],
                                     p20m[:])
                tsum = ep.tile([128, 1], F32)
                nc.vector.tensor_reduce(tsum[:], ptok[:],
                                        axis=mybir.AxisListType.X,
                                        op=ALU.add)
                lps = epp.tile([1, 1], F32)
                nc.tensor.matmul(lps[:], ones_col[:], tsum[:], start=True,
                                 stop=True)
                lsb = ep.tile([1, 1], F32)
                nc.vector.tensor_mul(lsb[:], lps[:], recn[:])
                nc.sync.dma_start(out[:], lsb[:])

    return nc


_NC = None


def _mask_layout(mask_row: np.ndarray) -> np.ndarray:
    # token t = 128 * col + row  ->  [128, ST]
    return np.ascontiguousarray(
        mask_row.astype(np.float32).reshape(ST, 128).T)


def kernel(last_hidden_states_1, last_hidden_states_2, token_mask_batch):
    global _NC
    h1 = np.ascontiguousarray(np.asarray(last_hidden_states_1,
                                         dtype=np.float32))
    h2 = np.ascontiguousarray(np.asarray(last_hidden_states_2,
                                         dtype=np.float32))
    mask = np.asarray(token_mask_batch)
    assert h1.shape == (NCORES, S, D), h1.shape

    if _NC is None:
        _NC = _build(NCORES)

    in_maps = [
        {"h1": h1[b], "h2": h2[b], "maskT": _mask_layout(mask[b])}
        for b in range(NCORES)
    ]
    res = run_bass_kernel_spmd(_NC, in_maps, list(range(NCORES)))
    losses = [float(np.asarray(res.results[b]["loss"]).reshape(()))
              for b in range(NCORES)]
    return np.float32(np.mean(losses))


# revision 16
# speedup vs baseline: 1.4086x; 1.2444x over previous
"""ContraCLM token-level contrastive loss on 8 Trainium2 NeuronCores.

Data-parallel over the batch: core b handles sample b (B=8). Per core,
with S=1536, D=1024, T=0.05, the 2S x 2S exp-sim row sums are built
from three quadrant families, exploiting the symmetry of the full
matrix (only ~70% of the blocks are computed):

  A = f1 f1^T upper triangle: row r covers cols [128r, 1536); row sums
      go to view-1 rows directly, column sums of the strictly-upper
      part are accumulated (DVE, bf16) into cac1 and folded back to
      view-1 rows at the end (they stand in for the mirrored lower
      triangle).
  C = f2 f1^T full rows (computed instead of B so each row block only
      needs one fT2 tile -> no wait on the full view-2 transpose):
      row sums to view-2 rows, column sums into cac1 (these are the
      B-quadrant contributions to view-1 rows).  The diagonal is the
      positive-pair similarity: it is extracted into poss20 via a
      DVE multiply with identity + reduce, and KEPT in the row sum
      (denom = Ng + pos needs exactly that).
  D = f2 f2^T upper triangle, like A, col sums into cac2.

  Self-similarity diagonals get -1e9 added in PSUM before exp -> exact
  zero contribution.  Masked tokens have f=0 (mask folded into the
  rsqrt scale), so each masked column adds exp(0)=1: subtract
  K0 = 2S - 2n.  per_tok = log(rowsum + K0') - pos_sim/T; masked mean;
  each core returns its per-sample mean and the host averages the 8
  scalars (no device collective).

  fp8e4 (x8) DoubleRow matmuls, K=1024 in 4 double-k groups.  exp row
  sums ride the ScalarE activation free-dim accumulator.  View-2 norms
  (sum of squares) run on GpSimd+DVE, interleaved with A' so the
  scalar queue stays clear for exps.
"""

import sys

for _p in ("/opt/trn_rl_repo", "/opt/pypackages"):
    if _p not in sys.path:
        sys.path.append(_p)

from contextlib import ExitStack

import numpy as np

import bass_rust

import concourse.bass as bass
import concourse.tile as tile
from concourse import mybir
from concourse.bass_utils import run_bass_kernel_spmd
from concourse.masks import make_identity
from concourse.vector_clock import ScopedClock

# The walrus build in this container encodes at most 2 sync waits per
# instruction (bass_rust's inst_waits_full agrees), but Tile's semaphore
# assignment can attach more. Hoist excess waits onto unfusable same-engine
# NoOps immediately before the instruction — the engine executes its queue
# in order, so semantics are preserved.
_MAX_WAITS = 1


def _split_excess_waits(nc, ordered):
    for bb_name, insts in ordered.items():
        out = []
        changed = False
        for inst in insts:
            si = getattr(inst, "sync_info", None)
            waits = list(si.on_wait) if si is not None else []
            if len(waits) > _MAX_WAITS:
                changed = True
                extra, keep = waits[:-_MAX_WAITS], waits[-_MAX_WAITS:]
                for i in range(0, len(extra), _MAX_WAITS):
                    out.append(mybir.InstNoOp(
                        name=nc.get_next_instruction_name(),
                        sync_info=mybir.SyncInfo(
                            on_wait=extra[i:i + _MAX_WAITS], on_update=[]),
                        bass_nofuse=True,
                        engine=inst.engine,
                    ))
                si.on_wait = keep
            out.append(inst)
        if changed:
            insts[:] = out


_orig_lower_ordered_insts = tile.TileContext._lower_ordered_insts


def _patched_lower_ordered_insts(self, ordered):
    _split_excess_waits(self.nc, ordered)
    return _orig_lower_ordered_insts(self, ordered)


tile.TileContext._lower_ordered_insts = _patched_lower_ordered_insts


def _split_waits_drain_and_barrier(self, tick_clock, wait_clock):
    nc = self.nc
    probe = nc.sync.nop(nofuse=True)
    wait_clock.add_sem_waits(
        probe.ins, ScopedClock({None: tick_clock.global_clock}))
    si = probe.ins.sync_info
    waits = list(si.on_wait) if si is not None else []
    if len(waits) > _MAX_WAITS:
        si.on_wait = waits[:_MAX_WAITS]
        for i in range(_MAX_WAITS, len(waits), _MAX_WAITS):
            nxt = nc.sync.nop(nofuse=True)
            nxt.ins.sync_info = bass_rust.SyncInfo(
                on_wait=waits[i:i + _MAX_WAITS], on_update=[])
    nc.sync.drain()
    nc.all_engine_barrier()
    assert self.sems is not None
    popped = nc._tile_sem_poison_stack.pop()
    assert popped is self._sem_poison
    nc.clear_and_free_semaphores(list(self.sems.allocated().values()))
    nc.all_engine_barrier()


tile.TileContext._drain_and_barrier = _split_waits_drain_and_barrier

S, D, NCORES = 1536, 1024, 8
ST = S // 128            # 12 s-tiles per view
NB = 2 * ST              # 24 block rows of F
KT = D // 128            # 8 contraction tiles
TEMP_INV = 20.0          # 1 / 0.05
FP8_SCALE = 8.0          # f entries ~N(0, 1/32); x8 keeps them in e4m3's
                         # normal range (|f|*8 <~ 2, well under 240)
EXP_SCALE = TEMP_INV / (FP8_SCALE * FP8_SCALE)
F32 = mybir.dt.float32
BF16 = mybir.dt.bfloat16
FP8 = mybir.dt.float8e4
AF = mybir.ActivationFunctionType
ALU = mybir.AluOpType
DR = mybir.MatmulPerfMode.DoubleRow


def _build(num_devices: int = NCORES, debug_dump: bool = False) -> bass.Bass:
    nc = bass.Bass(num_devices=num_devices)
    h1 = nc.dram_tensor("h1", [S, D], F32, kind="ExternalInput")
    h2 = nc.dram_tensor("h2", [S, D], F32, kind="ExternalInput")
    # mask, pre-laid-out host-side as [128, ST] so token t = 128*col + row
    maskT = nc.dram_tensor("maskT", [128, ST], F32, kind="ExternalInput")
    out = nc.dram_tensor("loss", [1, 1], F32, kind="ExternalOutput")
    if debug_dump:
        ng_dump = nc.dram_tensor("ng_dump", [128, NB], F32,
                                 kind="ExternalOutput")
        poss_dump = nc.dram_tensor("poss_dump", [128, ST], F32,
                                   kind="ExternalOutput")
        sc8_dump = nc.dram_tensor("sc8_dump", [128, NB], F32,
                                  kind="ExternalOutput")

    with tile.TileContext(nc) as tc, ExitStack() as ctx:
        const_pool = ctx.enter_context(tc.tile_pool(name="const", bufs=1))
        big = ctx.enter_context(tc.tile_pool(name="big", bufs=1))
        stat = ctx.enter_context(tc.tile_pool(name="stat", bufs=1))

        h1k = big.tile([128, ST, D], F32)
        h2k = big.tile([128, ST, D], F32)
        fT1 = big.tile([128, KT, S], FP8)        # f1^T * 8, fp8e4
        fT2 = big.tile([128, KT, S], FP8)        # f2^T * 8

        msk = const_pool.tile([128, ST], F32)
        # input DMAs first: they are the long pole at startup
        nc.sync.dma_start(msk[:], maskT[:])
        for t in range(ST):
            nc.sync.dma_start(h1k[:, t, :], h1[t * 128:(t + 1) * 128, :])
        for t in range(ST):
            nc.sync.dma_start(h2k[:, t, :], h2[t * 128:(t + 1) * 128, :])

        identF = const_pool.tile([128, 128], F32)
        make_identity(nc, identF[:])
        identB = const_pool.tile([128, 128], BF16)
        make_identity(nc, identB[:])
        # -1e9 on the diagonal, bf16: injected into self-sim PSUM blocks
        # via an extra accumulating matmul (identB^T @ negIB = -1e9 I)
        negIB = const_pool.tile([128, 128], BF16)
        nc.gpsimd.memset(negIB[:], 0.0)
        nc.gpsimd.affine_select(
            out=negIB[:], in_=negIB[:], compare_op=ALU.not_equal,
            fill=-1e9, base=0, pattern=[[-1, 128]], channel_multiplier=1)
        ones_col = const_pool.tile([128, 1], F32)
        nc.gpsimd.memset(ones_col[:], 1.0)
        ones_sq = const_pool.tile([128, 128], F32)
        nc.gpsimd.memset(ones_sq[:], 1.0)
        ones_bf = const_pool.tile([128, 1], BF16)
        nc.gpsimd.memset(ones_bf[:], 1.0)

        ss = stat.tile([128, NB], F32)           # per-token sum of squares
        sc8 = stat.tile([128, NB], F32)          # 8 * mask * rsqrt(ss)
        nrm = stat.tile([128, NB], F32)
        acc = stat.tile([128, NB, 2], F32)       # per-strip row sums
        cac1 = stat.tile([128, S], BF16)         # col acc -> view-1 rows
        cac2 = stat.tile([128, S], BF16)         # col acc -> view-2 rows
        poss20 = stat.tile([128, ST], F32)       # 64 * pos_sim
        msk24 = stat.tile([128, NB], F32)
        negK0 = stat.tile([128, 1], F32)
        recn = stat.tile([1, 1], F32)

        nc.gpsimd.memset(acc[:], 0.0)
        nc.gpsimd.memset(cac1[:], 0.0)
        nc.gpsimd.memset(cac2[:], 0.0)

        # ---- mask-only precomputes ----
        with tc.tile_pool(name="ep0", bufs=1) as ep0, \
             tc.tile_pool(name="ep0_ps", bufs=1, space="PSUM") as ep0p:
            msum = ep0.tile([128, 1], F32)
            nc.vector.tensor_reduce(msum[:], msk[:],
                                    axis=mybir.AxisListType.X, op=ALU.add)
            nps = ep0p.tile([128, 1], F32)
            nc.tensor.matmul(nps[:], ones_sq[:], msum[:], start=True,
                             stop=True)
            # -K0 = 2n - 2S
            nc.scalar.activation(negK0[:], nps[:], AF.Copy, scale=2.0,
                                 bias=float(-2 * S))
            n2c = ep0.tile([1, 1], F32)
            nc.scalar.activation(n2c[:], nps[0:1, :], AF.Copy, scale=2.0)
            nc.vector.reciprocal(recn[:], n2c[:])   # 1 / (2n)
            nc.vector.tensor_copy(msk24[:, 0:ST], msk[:])
            nc.vector.tensor_copy(msk24[:, ST:NB], msk[:])

        def finish_scale(o, n):
            """sc8[:, o:o+n] = 8 * msk * rsqrt(ss[:, o:o+n])."""
            nc.scalar.activation(nrm[:, o:o + n], ss[:, o:o + n], AF.Sqrt)
            ri = stat.tile([128, n], F32, name=f"ri_{o}")
            nc.vector.reciprocal(ri[:], nrm[:, o:o + n])
            rm = stat.tile([128, n], F32, name=f"rm_{o}")
            nc.vector.tensor_mul(rm[:], ri[:], msk24[:, o:o + n])
            nc.vector.tensor_scalar_mul(sc8[:, o:o + n], rm[:], FP8_SCALE)

        def transpose_tile(hk, fT, half, t, tps, scr, kg1_scalar):
            """fT[:, :, t*128:+128] = (hk[:,t,:] * sc8)^T as fp8."""
            o = half * ST
            fn = scr.tile([128, D], BF16, tag="fn", name=f"fn_{half}_{t}")
            nc.vector.tensor_scalar_mul(fn[:], hk[:, t, :],
                                        sc8[:, o + t:o + t + 1])
            c0 = t * 128
            for kg in range(2):
                pt = tps.tile([128, 512], BF16, tag="pt",
                              name=f"pt_{half}_{t}_{kg}")
                for j in range(4):
                    k = kg * 4 + j
                    nc.tensor.transpose(pt[:, j * 128:(j + 1) * 128],
                                        fn[:, k * 128:(k + 1) * 128],
                                        identB[:])
                dst = fT[:, kg * 4:(kg + 1) * 4, c0:c0 + 128]
                src = pt[:].rearrange("p (j c) -> p j c", j=4)
                if kg == 1 and kg1_scalar:
                    nc.scalar.copy(dst, src)
                else:
                    nc.vector.tensor_copy(dst, src)

        def mm_strip(ps, lhsT, rT, rhsT, col0, ncols):
            """sim strip into ps[:, 0:ncols] (DoubleRow, K=1024)."""
            for g in range(KT // 2):
                u0 = 0
                while u0 < ncols:
                    u1 = min(u0 + 512, ncols)
                    nc.tensor.matmul(
                        ps[:, u0:u1],
                        lhsT[:, 2 * g:2 * g + 2, rT * 128:(rT + 1) * 128],
                        rhsT[:, 2 * g:2 * g + 2, col0 + u0:col0 + u1],
                        perf_mode=DR,
                        start=(g == 0), stop=(g == KT // 2 - 1))
                    u0 = u1

        # ---- phase A: view-1 norms (scalar) + transpose, in halves ----
        with tc.tile_pool(name="sqpA", bufs=2) as sqp, \
             tc.tile_pool(name="scrA", bufs=2) as scr, \
             tc.tile_pool(name="tpA_ps", bufs=2, space="PSUM") as tps:
            for hf in range(2):
                t0 = hf * (ST // 2)
                for t in range(t0, t0 + ST // 2):
                    sq = sqp.tile([128, D], BF16, tag="sq", name=f"sqA_{t}")
                    nc.scalar.activation(sq[:], h1k[:, t, :], AF.Square,
                                         accum_out=ss[:, t:t + 1])
                finish_scale(t0, ST // 2)
                for t in range(t0, t0 + ST // 2):
                    transpose_tile(h1k, fT1, 0, t, tps, scr,
                                   kg1_scalar=True)

        # ---- phase A': A-quadrant upper-triangle strips, interleaved
        # with view-2 norms (GpSimd squares + DVE reduces) ----
        with tc.tile_pool(name="psA", bufs=2, space="PSUM") as psA, \
             tc.tile_pool(name="esA", bufs=2) as esA, \
             tc.tile_pool(name="sqp2", bufs=2) as sqp2:
            for r in range(ST):
                # view-2 norm work for tile r rides along
                sq2 = sqp2.tile([128, D], BF16, tag="sq2", name=f"sq2_{r}")
                nc.gpsimd.tensor_mul(sq2[:], h2k[:, r, :], h2k[:, r, :])
                nc.vector.tensor_reduce(ss[:, ST + r:ST + r + 1], sq2[:],
                                        axis=mybir.AxisListType.X,
                                        op=ALU.add)

                ncols = S - r * 128
                trip = psA.tile([128, S], F32, tag="tp", name=f"tpA_{r}")
                mm_strip(trip, fT1, r, fT1, r * 128, ncols)
                # self diagonal (block 0 of this strip) -> -1e9 before exp
                nc.tensor.matmul(trip[:, 0:128], identB[:], negIB[:],
                                 start=False, stop=True,
                                 skip_group_check=True)
                es = esA.tile([128, S], BF16, tag="es", name=f"esA_{r}")
                nc.scalar.activation(es[:, 0:ncols], trip[:, 0:ncols],
                                     AF.Exp, scale=EXP_SCALE,
                                     accum_out=acc[:, r, 0:1])
                if ncols > 128:
                    # strictly-upper cols mirror into view-1 rows
                    nc.vector.tensor_add(cac1[:, (r + 1) * 128:S],
                                         cac1[:, (r + 1) * 128:S],
                                         es[:, 128:ncols])
            finish_scale(ST, ST)

        # ---- phase B: view-2 transpose ----
        with tc.tile_pool(name="scrB", bufs=2) as scr, \
             tc.tile_pool(name="tpB_ps", bufs=2, space="PSUM") as tps:
            for t in range(ST):
                transpose_tile(h2k, fT2, 1, t, tps, scr, kg1_scalar=True)

        # ---- phase B': C rows then D upper-triangle rows ----
        with tc.tile_pool(name="psB", bufs=2, space="PSUM") as psB, \
             tc.tile_pool(name="cb_ps", bufs=1, space="PSUM") as cbp, \
             tc.tile_pool(name="esB", bufs=2) as esB, \
             tc.tile_pool(name="dvB", bufs=2) as dvB:
            pcbt = cbp.tile([128, 2, ST], F32, name="pcbt")
            # C = f2 f1^T: row block rT only needs fT2 tile rT
            for rT in range(ST):
                trip = psB.tile([128, S], F32, tag="tp", name=f"tpC_{rT}")
                mm_strip(trip, fT2, rT, fT1, 0, S)
                # counterpart diagonal: extract 64*pos_sim, keep in sum
                dscr = dvB.tile([128, 128], F32, tag="dg", name=f"dg_{rT}")
                nc.vector.tensor_mul(
                    dscr[:], trip[:, rT * 128:(rT + 1) * 128], identF[:])
                nc.vector.tensor_reduce(
                    poss20[:, rT:rT + 1], dscr[:],
                    axis=mybir.AxisListType.X, op=ALU.add)
                es = esB.tile([128, S], BF16, tag="es", name=f"esC_{rT}")
                nc.scalar.activation(es[:], trip[:], AF.Exp,
                                     scale=EXP_SCALE,
                                     accum_out=acc[:, ST + rT, 0:1])
                nc.vector.tensor_add(cac1[:], cac1[:], es[:])
                if rT == ST - 1:
                    # fold cac1 partitions -> view-1 row contributions
                    for jb in range(ST):
                        nc.tensor.matmul(
                            pcbt[:, 0, jb:jb + 1],
                            cac1[:, jb * 128:(jb + 1) * 128],
                            ones_bf[:], start=True, stop=True,
                            skip_group_check=True)
            # D = f2 f2^T upper triangle
            for rT in range(ST):
                ncols = S - rT * 128
                trip = psB.tile([128, S], F32, tag="tp", name=f"tpD_{rT}")
                mm_strip(trip, fT2, rT, fT2, rT * 128, ncols)
                nc.tensor.matmul(trip[:, 0:128], identB[:], negIB[:],
                                 start=False, stop=True,
                                 skip_group_check=True)
                es = esB.tile([128, S], BF16, tag="es", name=f"esD_{rT}")
                nc.scalar.activation(es[:, 0:ncols], trip[:, 0:ncols],
                                     AF.Exp, scale=EXP_SCALE,
                                     accum_out=acc[:, ST + rT, 1:2])
                if ncols > 128:
                    nc.vector.tensor_add(cac2[:, (rT + 1) * 128:S],
                                         cac2[:, (rT + 1) * 128:S],
                                         es[:, 128:ncols])
                if rT == ST - 1:
                    for jb in range(ST):
                        nc.tensor.matmul(
                            pcbt[:, 1, jb:jb + 1],
                            cac2[:, jb * 128:(jb + 1) * 128],
                            ones_bf[:], start=True, stop=True,
                            skip_group_check=True)

            # ---- epilogue ----
            with tc.tile_pool(name="ep", bufs=1) as ep, \
                 tc.tile_pool(name="ep_ps", bufs=1, space="PSUM") as epp:
                ng = ep.tile([128, NB], F32)
                nc.vector.tensor_reduce(ng[:], acc[:],
                                        axis=mybir.AxisListType.X,
                                        op=ALU.add)
                nc.vector.tensor_add(ng[:, 0:ST], ng[:, 0:ST], pcbt[:, 0, :])
                nc.vector.tensor_add(ng[:, ST:NB], ng[:, ST:NB], pcbt[:, 1, :])
                denom = ep.tile([128, NB], F32)
                nc.vector.tensor_scalar_add(denom[:], ng[:], negK0[:])
                lg = ep.tile([128, NB], F32)
                nc.scalar.activation(lg[:], denom[:], AF.Ln)
                if debug_dump:
                    nc.sync.dma_start(ng_dump[:], ng[:])
                    nc.sync.dma_start(poss_dump[:], poss20[:])
                    nc.sync.dma_start(sc8_dump[:], sc8[:])
                ptok = ep.tile([128, NB], F32)
                nc.vector.tensor_mul(ptok[:], lg[:], msk24[:])
                p20m = ep.tile([128, ST], F32)
                nc.vector.tensor_mul(p20m[:], poss20[:], msk[:])
                # poss20 held 64*pos_sim (raw psum); scale to pos_sim/T
                nc.vector.tensor_scalar_mul(p20m[:], p20m[:], EXP_SCALE)
                nc.vector.tensor_sub(ptok[:, 0:ST], ptok[:, 0:ST], p20m[:])
                nc.vector.tensor_sub(ptok[:, ST:NB], ptok[:, ST:NB],
                                     p20m[:])
                tsum = ep.tile([128, 1], F32)
                nc.vector.tensor_reduce(tsum[:], ptok[:],
                                        axis=mybir.AxisListType.X,
                                        op=ALU.add)
                lps = epp.tile([1, 1], F32)
                nc.tensor.matmul(lps[:], ones_col[:], tsum[:], start=True,
                                 stop=True)
                lsb = ep.tile([1, 1], F32)
                nc.vector.tensor_mul(lsb[:], lps[:], recn[:])
                nc.sync.dma_start(out[:], lsb[:])

    return nc


_NC = None


def _mask_layout(mask_row: np.ndarray) -> np.ndarray:
    # token t = 128 * col + row  ->  [128, ST]
    return np.ascontiguousarray(
        mask_row.astype(np.float32).reshape(ST, 128).T)


def kernel(last_hidden_states_1, last_hidden_states_2, token_mask_batch):
    global _NC
    h1 = np.ascontiguousarray(np.asarray(last_hidden_states_1,
                                         dtype=np.float32))
    h2 = np.ascontiguousarray(np.asarray(last_hidden_states_2,
                                         dtype=np.float32))
    mask = np.asarray(token_mask_batch)
    assert h1.shape == (NCORES, S, D), h1.shape

    if _NC is None:
        _NC = _build(NCORES)

    in_maps = [
        {"h1": h1[b], "h2": h2[b], "maskT": _mask_layout(mask[b])}
        for b in range(NCORES)
    ]
    res = run_bass_kernel_spmd(_NC, in_maps, list(range(NCORES)))
    losses = [float(np.asarray(res.results[b]["loss"]).reshape(()))
              for b in range(NCORES)]
    return np.float32(np.mean(losses))


# revision 18
# speedup vs baseline: 1.4584x; 1.0353x over previous
"""ContraCLM token-level contrastive loss on 8 Trainium2 NeuronCores.

Data-parallel over the batch: core b handles sample b (B=8). Per core,
with S=1536, D=1024, T=0.05, the 2S x 2S exp-sim row sums are built
from three quadrant families, exploiting the symmetry of the full
matrix (only ~70% of the blocks are computed):

  A = f1 f1^T upper triangle: row r covers cols [128r, 1536); row sums
      go to view-1 rows directly, column sums of the strictly-upper
      part are accumulated (DVE, bf16) into cac1 and folded back to
      view-1 rows at the end (they stand in for the mirrored lower
      triangle).
  C = f2 f1^T full rows (computed instead of B so each row block only
      needs one fT2 tile -> no wait on the full view-2 transpose):
      row sums to view-2 rows, column sums into cac1 (these are the
      B-quadrant contributions to view-1 rows).  The diagonal is the
      positive-pair similarity: it is extracted into poss20 via a
      DVE multiply with identity + reduce, and KEPT in the row sum
      (denom = Ng + pos needs exactly that).
  D = f2 f2^T upper triangle, like A, col sums into cac2.

  Self-similarity diagonals get -1e9 added in PSUM before exp -> exact
  zero contribution.  Masked tokens have f=0 (mask folded into the
  rsqrt scale), so each masked column adds exp(0)=1: subtract
  K0 = 2S - 2n.  per_tok = log(rowsum + K0') - pos_sim/T; masked mean;
  each core returns its per-sample mean and the host averages the 8
  scalars (no device collective).

  fp8e4 (x8) DoubleRow matmuls, K=1024 in 4 double-k groups.  exp row
  sums ride the ScalarE activation free-dim accumulator.  View-2 norms
  (sum of squares) run on GpSimd+DVE, interleaved with A' so the
  scalar queue stays clear for exps.
"""

import sys

for _p in ("/opt/trn_rl_repo", "/opt/pypackages"):
    if _p not in sys.path:
        sys.path.append(_p)

from contextlib import ExitStack

import numpy as np

import bass_rust

import concourse.bass as bass
import concourse.tile as tile
from concourse import mybir
from concourse.bass_utils import run_bass_kernel_spmd
from concourse.masks import make_identity
from concourse.vector_clock import ScopedClock

# The walrus build in this container encodes at most 2 sync waits per
# instruction (bass_rust's inst_waits_full agrees), but Tile's semaphore
# assignment can attach more. Hoist excess waits onto unfusable same-engine
# NoOps immediately before the instruction — the engine executes its queue
# in order, so semantics are preserved.
_MAX_WAITS = 1


def _split_excess_waits(nc, ordered):
    for bb_name, insts in ordered.items():
        out = []
        changed = False
        for inst in insts:
            si = getattr(inst, "sync_info", None)
            waits = list(si.on_wait) if si is not None else []
            if len(waits) > _MAX_WAITS:
                changed = True
                extra, keep = waits[:-_MAX_WAITS], waits[-_MAX_WAITS:]
                for i in range(0, len(extra), _MAX_WAITS):
                    out.append(mybir.InstNoOp(
                        name=nc.get_next_instruction_name(),
                        sync_info=mybir.SyncInfo(
                            on_wait=extra[i:i + _MAX_WAITS], on_update=[]),
                        bass_nofuse=True,
                        engine=inst.engine,
                    ))
                si.on_wait = keep
            out.append(inst)
        if changed:
            insts[:] = out


_orig_lower_ordered_insts = tile.TileContext._lower_ordered_insts


def _patched_lower_ordered_insts(self, ordered):
    _split_excess_waits(self.nc, ordered)
    return _orig_lower_ordered_insts(self, ordered)


tile.TileContext._lower_ordered_insts = _patched_lower_ordered_insts


def _split_waits_drain_and_barrier(self, tick_clock, wait_clock):
    nc = self.nc
    probe = nc.sync.nop(nofuse=True)
    wait_clock.add_sem_waits(
        probe.ins, ScopedClock({None: tick_clock.global_clock}))
    si = probe.ins.sync_info
    waits = list(si.on_wait) if si is not None else []
    if len(waits) > _MAX_WAITS:
        si.on_wait = waits[:_MAX_WAITS]
        for i in range(_MAX_WAITS, len(waits), _MAX_WAITS):
            nxt = nc.sync.nop(nofuse=True)
            nxt.ins.sync_info = bass_rust.SyncInfo(
                on_wait=waits[i:i + _MAX_WAITS], on_update=[])
    nc.sync.drain()
    nc.all_engine_barrier()
    assert self.sems is not None
    popped = nc._tile_sem_poison_stack.pop()
    assert popped is self._sem_poison
    nc.clear_and_free_semaphores(list(self.sems.allocated().values()))
    nc.all_engine_barrier()


tile.TileContext._drain_and_barrier = _split_waits_drain_and_barrier

S, D, NCORES = 1536, 1024, 8
ST = S // 128            # 12 s-tiles per view
NB = 2 * ST              # 24 block rows of F
KT = D // 128            # 8 contraction tiles
TEMP_INV = 20.0          # 1 / 0.05
FP8_SCALE = 8.0          # f entries ~N(0, 1/32); x8 keeps them in e4m3's
                         # normal range (|f|*8 <~ 2, well under 240)
EXP_SCALE = TEMP_INV / (FP8_SCALE * FP8_SCALE)
F32 = mybir.dt.float32
BF16 = mybir.dt.bfloat16
FP8 = mybir.dt.float8e4
AF = mybir.ActivationFunctionType
ALU = mybir.AluOpType
DR = mybir.MatmulPerfMode.DoubleRow


def _build(num_devices: int = NCORES, debug_dump: bool = False) -> bass.Bass:
    nc = bass.Bass(num_devices=num_devices)
    h1 = nc.dram_tensor("h1", [S, D], F32, kind="ExternalInput")
    h2 = nc.dram_tensor("h2", [S, D], F32, kind="ExternalInput")
    # mask, pre-laid-out host-side as [128, ST] so token t = 128*col + row
    maskT = nc.dram_tensor("maskT", [128, ST], F32, kind="ExternalInput")
    out = nc.dram_tensor("loss", [1, 1], F32, kind="ExternalOutput")
    if debug_dump:
        ng_dump = nc.dram_tensor("ng_dump", [128, NB], F32,
                                 kind="ExternalOutput")
        poss_dump = nc.dram_tensor("poss_dump", [128, ST], F32,
                                   kind="ExternalOutput")
        sc8_dump = nc.dram_tensor("sc8_dump", [128, NB], F32,
                                  kind="ExternalOutput")

    with tile.TileContext(nc) as tc, ExitStack() as ctx:
        const_pool = ctx.enter_context(tc.tile_pool(name="const", bufs=1))
        big = ctx.enter_context(tc.tile_pool(name="big", bufs=1))
        stat = ctx.enter_context(tc.tile_pool(name="stat", bufs=1))

        h1k = big.tile([128, ST, D], F32)
        h2k = big.tile([128, ST, D], F32)
        fT1 = big.tile([128, KT, S], FP8)        # f1^T * 8, fp8e4
        fT2 = big.tile([128, KT, S], FP8)        # f2^T * 8

        msk = const_pool.tile([128, ST], F32)
        # input DMAs first: they are the long pole at startup
        nc.sync.dma_start(msk[:], maskT[:])
        for t in range(ST):
            nc.sync.dma_start(h1k[:, t, :], h1[t * 128:(t + 1) * 128, :])
        for t in range(ST):
            nc.sync.dma_start(h2k[:, t, :], h2[t * 128:(t + 1) * 128, :])

        identF = const_pool.tile([128, 128], F32)
        make_identity(nc, identF[:])
        identB = const_pool.tile([128, 128], BF16)
        make_identity(nc, identB[:])
        # -1e9 on the diagonal, bf16: injected into self-sim PSUM blocks
        # via an extra accumulating matmul (identB^T @ negIB = -1e9 I)
        negIB = const_pool.tile([128, 128], BF16)
        nc.gpsimd.memset(negIB[:], 0.0)
        nc.gpsimd.affine_select(
            out=negIB[:], in_=negIB[:], compare_op=ALU.not_equal,
            fill=-1e9, base=0, pattern=[[-1, 128]], channel_multiplier=1)
        ones_col = const_pool.tile([128, 1], F32)
        nc.gpsimd.memset(ones_col[:], 1.0)
        ones_sq = const_pool.tile([128, 128], F32)
        nc.gpsimd.memset(ones_sq[:], 1.0)
        ones_bf = const_pool.tile([128, 1], BF16)
        nc.gpsimd.memset(ones_bf[:], 1.0)

        ss = stat.tile([128, NB], F32)           # per-token sum of squares
        sc8 = stat.tile([128, NB], F32)          # 8 * mask * rsqrt(ss)
        nrm = stat.tile([128, NB], F32)
        acc = stat.tile([128, NB, 2], F32)       # per-strip row sums
        cac1 = stat.tile([128, S], BF16)         # col acc -> view-1 rows
        cac2 = stat.tile([128, S], BF16)         # col acc -> view-2 rows
        poss20 = stat.tile([128, ST], F32)       # 64 * pos_sim
        msk24 = stat.tile([128, NB], F32)
        negK0 = stat.tile([128, 1], F32)
        recn = stat.tile([1, 1], F32)

        nc.gpsimd.memset(acc[:], 0.0)
        nc.gpsimd.memset(cac1[:], 0.0)
        nc.gpsimd.memset(cac2[:], 0.0)

        # ---- mask-only precomputes ----
        with tc.tile_pool(name="ep0", bufs=1) as ep0, \
             tc.tile_pool(name="ep0_ps", bufs=1, space="PSUM") as ep0p:
            msum = ep0.tile([128, 1], F32)
            nc.vector.tensor_reduce(msum[:], msk[:],
                                    axis=mybir.AxisListType.X, op=ALU.add)
            nps = ep0p.tile([128, 1], F32)
            nc.tensor.matmul(nps[:], ones_sq[:], msum[:], start=True,
                             stop=True)
            # -K0 = 2n - 2S
            nc.scalar.activation(negK0[:], nps[:], AF.Copy, scale=2.0,
                                 bias=float(-2 * S))
            n2c = ep0.tile([1, 1], F32)
            nc.scalar.activation(n2c[:], nps[0:1, :], AF.Copy, scale=2.0)
            nc.vector.reciprocal(recn[:], n2c[:])   # 1 / (2n)
            nc.vector.tensor_copy(msk24[:, 0:ST], msk[:])
            nc.vector.tensor_copy(msk24[:, ST:NB], msk[:])

        def finish_scale(o, n):
            """sc8[:, o:o+n] = 8 * msk * rsqrt(ss[:, o:o+n])."""
            nc.scalar.activation(nrm[:, o:o + n], ss[:, o:o + n], AF.Sqrt)
            ri = stat.tile([128, n], F32, name=f"ri_{o}")
            nc.vector.reciprocal(ri[:], nrm[:, o:o + n])
            rm = stat.tile([128, n], F32, name=f"rm_{o}")
            nc.vector.tensor_mul(rm[:], ri[:], msk24[:, o:o + n])
            nc.vector.tensor_scalar_mul(sc8[:, o:o + n], rm[:], FP8_SCALE)

        def transpose_tile(hk, fT, half, t, tps, scr, kg1_scalar):
            """fT[:, :, t*128:+128] = (hk[:,t,:] * sc8)^T as fp8."""
            o = half * ST
            fn = scr.tile([128, D], BF16, tag="fn", name=f"fn_{half}_{t}")
            nc.vector.tensor_scalar_mul(fn[:], hk[:, t, :],
                                        sc8[:, o + t:o + t + 1])
            c0 = t * 128
            for kg in range(2):
                pt = tps.tile([128, 512], BF16, tag="pt",
                              name=f"pt_{half}_{t}_{kg}")
                for j in range(4):
                    k = kg * 4 + j
                    nc.tensor.transpose(pt[:, j * 128:(j + 1) * 128],
                                        fn[:, k * 128:(k + 1) * 128],
                                        identB[:])
                dst = fT[:, kg * 4:(kg + 1) * 4, c0:c0 + 128]
                src = pt[:].rearrange("p (j c) -> p j c", j=4)
                if kg == 1 and kg1_scalar:
                    nc.scalar.copy(dst, src)
                else:
                    nc.vector.tensor_copy(dst, src)

        def mm_strip(ps, lhsT, rT, rhsT, col0, ncols):
            """sim strip into ps[:, 0:ncols] (DoubleRow, K=1024)."""
            for g in range(KT // 2):
                u0 = 0
                while u0 < ncols:
                    u1 = min(u0 + 512, ncols)
                    nc.tensor.matmul(
                        ps[:, u0:u1],
                        lhsT[:, 2 * g:2 * g + 2, rT * 128:(rT + 1) * 128],
                        rhsT[:, 2 * g:2 * g + 2, col0 + u0:col0 + u1],
                        perf_mode=DR,
                        start=(g == 0), stop=(g == KT // 2 - 1))
                    u0 = u1

        # ---- phase A: view-1 norms (scalar) + transpose, in halves;
        # view-2 squares (GpSimd) ride along as h2 tiles land ----
        sq2b = big.tile([128, ST, D], BF16)
        with tc.tile_pool(name="sqpA", bufs=2) as sqp, \
             tc.tile_pool(name="scrA", bufs=3) as scr, \
             tc.tile_pool(name="tpA_ps", bufs=2, space="PSUM") as tps:
            for hf in range(2):
                t0 = hf * (ST // 2)
                for t in range(t0, t0 + ST // 2):
                    sq = sqp.tile([128, D], BF16, tag="sq", name=f"sqA_{t}")
                    nc.scalar.activation(sq[:], h1k[:, t, :], AF.Square,
                                         accum_out=ss[:, t:t + 1])
                finish_scale(t0, ST // 2)
                for t in range(t0, t0 + ST // 2):
                    transpose_tile(h1k, fT1, 0, t, tps, scr,
                                   kg1_scalar=True)
                    nc.gpsimd.tensor_mul(sq2b[:, t, :], h2k[:, t, :],
                                         h2k[:, t, :])

        # view-2 norms finish on DVE before A' so sc8_2 is ready early
        for t in range(ST):
            nc.vector.tensor_reduce(ss[:, ST + t:ST + t + 1],
                                    sq2b[:, t, :],
                                    axis=mybir.AxisListType.X, op=ALU.add)
        finish_scale(ST, ST)

        # ---- A' (A-quadrant upper triangle), phase-B transposes and
        # C rows interleaved to keep TensorE continuously busy ----
        with ExitStack() as bctx:
            psA = bctx.enter_context(
                tc.tile_pool(name="psA", bufs=2, space="PSUM"))
            esA = bctx.enter_context(tc.tile_pool(name="esA", bufs=2))
            scrB = bctx.enter_context(tc.tile_pool(name="scrB", bufs=3))
            dvB = bctx.enter_context(tc.tile_pool(name="dvB", bufs=2))
            if True:

                def a_row(r):
                    ncols = S - r * 128
                    trip = psA.tile([128, S], F32, tag="tp",
                                    name=f"tpA_{r}")
                    mm_strip(trip, fT1, r, fT1, r * 128, ncols)
                    nc.tensor.matmul(trip[:, 0:128], identB[:], negIB[:],
                                     start=False, stop=True,
                                     skip_group_check=True)
                    es = esA.tile([128, S], BF16, tag="es",
                                  name=f"esA_{r}")
                    nc.scalar.activation(es[:, 0:ncols], trip[:, 0:ncols],
                                         AF.Exp, scale=EXP_SCALE,
                                         accum_out=acc[:, r, 0:1])
                    if ncols > 128:
                        nc.gpsimd.tensor_add(cac1[:, (r + 1) * 128:S],
                                             cac1[:, (r + 1) * 128:S],
                                             es[:, 128:ncols])

                def c_row(rT):
                    trip = psA.tile([128, S], F32, tag="tp",
                                    name=f"tpC_{rT}")
                    mm_strip(trip, fT2, rT, fT1, 0, S)
                    # counterpart diagonal: extract 64*pos_sim, keep it
                    # inside the row sum (denom = Ng + pos)
                    dscr = dvB.tile([128, 128], F32, tag="dg",
                                    name=f"dg_{rT}")
                    nc.vector.tensor_mul(
                        dscr[:], trip[:, rT * 128:(rT + 1) * 128],
                        identF[:])
                    nc.vector.tensor_reduce(
                        poss20[:, rT:rT + 1], dscr[:],
                        axis=mybir.AxisListType.X, op=ALU.add)
                    es = esA.tile([128, S], BF16, tag="es",
                                  name=f"esC_{rT}")
                    nc.scalar.activation(es[:], trip[:], AF.Exp,
                                         scale=EXP_SCALE,
                                         accum_out=acc[:, ST + rT, 0:1])
                    nc.vector.tensor_add(cac1[:], cac1[:], es[:])

                def d_row(rT):
                    ncols = S - rT * 128
                    trip = psA.tile([128, S], F32, tag="tp",
                                    name=f"tpD_{rT}")
                    mm_strip(trip, fT2, rT, fT2, rT * 128, ncols)
                    nc.tensor.matmul(trip[:, 0:128], identB[:], negIB[:],
                                     start=False, stop=True,
                                     skip_group_check=True)
                    es = esA.tile([128, S], BF16, tag="es",
                                  name=f"esD_{rT}")
                    nc.scalar.activation(es[:, 0:ncols], trip[:, 0:ncols],
                                         AF.Exp, scale=EXP_SCALE,
                                         accum_out=acc[:, ST + rT, 1:2])
                    if ncols > 128:
                        nc.gpsimd.tensor_add(cac2[:, (rT + 1) * 128:S],
                                             cac2[:, (rT + 1) * 128:S],
                                             es[:, 128:ncols])

                def fold(cac, half):
                    for jb in range(ST):
                        nc.tensor.matmul(
                            pcbt[:, half, jb:jb + 1],
                            cac[:, jb * 128:(jb + 1) * 128],
                            ones_bf[:], start=True, stop=True,
                            skip_group_check=True)

                with tc.tile_pool(name="tpB_ps", bufs=2,
                                  space="PSUM") as tpsB:
                    for r in range(6):
                        a_row(r)
                    for r in range(6, ST):
                        a_row(r)
                        transpose_tile(h2k, fT2, 1, r - 6, tpsB, scrB,
                                       kg1_scalar=True)
                    for i in range(6):
                        transpose_tile(h2k, fT2, 1, 6 + i, tpsB, scrB,
                                       kg1_scalar=True)
                        c_row(i)
                cbp = bctx.enter_context(
                    tc.tile_pool(name="cb_ps", bufs=1, space="PSUM"))
                pcbt = cbp.tile([128, 2, ST], F32, name="pcbt")
                for rT in range(6, ST):
                    c_row(rT)
                d_row(0)
                d_row(1)
                d_row(2)
                fold(cac1, 0)
                for rT in range(3, ST):
                    d_row(rT)
                fold(cac2, 1)

            # ---- epilogue ----
            with tc.tile_pool(name="ep", bufs=1) as ep, \
                 tc.tile_pool(name="ep_ps", bufs=1, space="PSUM") as epp:
                ng = ep.tile([128, NB], F32)
                nc.vector.tensor_reduce(ng[:], acc[:],
                                        axis=mybir.AxisListType.X,
                                        op=ALU.add)
                nc.vector.tensor_add(ng[:, 0:ST], ng[:, 0:ST],
                                     pcbt[:, 0, :])
                nc.vector.tensor_add(ng[:, ST:NB], ng[:, ST:NB],
                                     pcbt[:, 1, :])
                denom = ep.tile([128, NB], F32)
                nc.vector.tensor_scalar_add(denom[:], ng[:], negK0[:])
                lg = ep.tile([128, NB], F32)
                nc.scalar.activation(lg[:], denom[:], AF.Ln)
                if debug_dump:
                    nc.sync.dma_start(ng_dump[:], ng[:])
                    nc.sync.dma_start(poss_dump[:], poss20[:])
                    nc.sync.dma_start(sc8_dump[:], sc8[:])
                ptok = ep.tile([128, NB], F32)
                nc.vector.tensor_mul(ptok[:], lg[:], msk24[:])
                p20m = ep.tile([128, ST], F32)
                nc.vector.tensor_mul(p20m[:], poss20[:], msk[:])
                # poss20 held 64*pos_sim (raw psum); scale to pos_sim/T
                nc.vector.tensor_scalar_mul(p20m[:], p20m[:], EXP_SCALE)
                nc.vector.tensor_sub(ptok[:, 0:ST], ptok[:, 0:ST],
                                     p20m[:])
                nc.vector.tensor_sub(ptok[:, ST:NB], ptok[:, ST:NB],
                                     p20m[:])
                tsum = ep.tile([128, 1], F32)
                nc.vector.tensor_reduce(tsum[:], ptok[:],
                                        axis=mybir.AxisListType.X,
                                        op=ALU.add)
                lps = epp.tile([1, 1], F32)
                nc.tensor.matmul(lps[:], ones_col[:], tsum[:], start=True,
                                 stop=True)
                lsb = ep.tile([1, 1], F32)
                nc.vector.tensor_mul(lsb[:], lps[:], recn[:])
                nc.sync.dma_start(out[:], lsb[:])

    return nc


_NC = None


def _mask_layout(mask_row: np.ndarray) -> np.ndarray:
    # token t = 128 * col + row  ->  [128, ST]
    return np.ascontiguousarray(
        mask_row.astype(np.float32).reshape(ST, 128).T)


def kernel(last_hidden_states_1, last_hidden_states_2, token_mask_batch):
    global _NC
    h1 = np.ascontiguousarray(np.asarray(last_hidden_states_1,
                                         dtype=np.float32))
    h2 = np.ascontiguousarray(np.asarray(last_hidden_states_2,
                                         dtype=np.float32))
    mask = np.asarray(token_mask_batch)
    assert h1.shape == (NCORES, S, D), h1.shape

    if _NC is None:
        _NC = _build(NCORES)

    in_maps = [
        {"h1": h1[b], "h2": h2[b], "maskT": _mask_layout(mask[b])}
        for b in range(NCORES)
    ]
    res = run_bass_kernel_spmd(_NC, in_maps, list(range(NCORES)))
    losses = [float(np.asarray(res.results[b]["loss"]).reshape(()))
              for b in range(NCORES)]
    return np.float32(np.mean(losses))
